# revision 1
# baseline (speedup 1.0000x reference)
"""FOFEReader Trainium2 kernel: 8-core SPMD (batch x s-half sharding).

Math (per batch b, candidate (s, e=s+j), j<16):
  F[t] = sum_{k<=t} a^(t-k) doc[k]      (prefix FOFE),  R[t] = sum_{k>=t} a^(k-t) doc[k]
  x = [F[s-1] | F[s+j] - a^(j+1) F[s-1] | R[s+j+1] | qf]
  out = (relu(bn2(relu(bn1(x @ W1.T)) @ W2.T)) @ W3.T)
Reformulated so the 1212-dim GEMM is shared across the 16 spans j:
  G_u = U_u @ F (u in {l,c}), G_r = U_r @ R   with W1.T = [U_l U_c U_r U_q] row blocks
  z1[s,j] = (G_l[s-1] + q1) + (G_c[s+j] + G_r[s+j+1]) - a^(j+1) G_c[s-1]
The bn1 scale is folded into the G tensors at PSUM eviction (ScalarE per-partition
scale), so the span loop is two adds + a scalar-multiply + a plain relu per tile.
F/R and G GEMMs run in fp32r (TF32); the big 1024x512 layer-2 GEMM runs in bf16
(full PE rate, half LDWEIGHTS cost). The span-loop elementwise chain runs in bf16
on VectorE for 6 of 8 h-tiles and in fp32 on GpSimd for the other 2 (GpSimd is
~2x slower and unusable for bf16, but otherwise idle).
Each core handles one batch and one half of the s range (406 starts + halo).
"""
import os
import sys

for _p in ("/opt/trn_rl_repo", "/root/.axon_site/_ro/trn_rl_repo"):
    if os.path.isdir(_p) and _p not in sys.path:
        sys.path.insert(0, _p)
        break

import numpy as np

T = 809
MSPAN = 16
B = 4
ALPHA = 0.9
NS = 406          # s-starts per core (even: f32r matmul needs even free dim)
WIN = 424         # t window per core: t = s_lo-1 + i, i in [0, 424)
DD = 304
EMB = 300
LQ = 30
H4 = 1024
H2 = 512
BN_EPS = 1e-5
N_CORES = 8
NHT = H4 // 128   # 8
NMT = H2 // 128   # 4
N_F32_HT = 0      # h-tiles assembled in fp32 on GpSimd (rest: bf16 on VectorE)

_CACHE = {}


def _round_tf32(a):
    a = np.ascontiguousarray(a, dtype=np.float32)
    return (a.view(np.uint32) & np.uint32(0xFFFFE000)).view(np.float32)


def _build_amat(s_lo):
    """[809, 2*WIN] fp32: cols 0..WIN-1 = forward-FOFE operator columns for
    t=s_lo-1+i (A^T slice), cols WIN.. = reverse. Out-of-range t -> zero col."""
    t_idx = s_lo - 1 + np.arange(WIN)
    kv = np.arange(T)[:, None]
    tv = t_idx[None, :]
    valid = ((t_idx >= 0) & (t_idx <= T - 1))[None, :]
    af = np.where((kv <= tv) & valid, ALPHA ** np.maximum(tv - kv, 0), 0.0)
    ar = np.where((kv >= tv) & valid, ALPHA ** np.maximum(kv - tv, 0), 0.0)
    return _round_tf32(np.concatenate([af, ar], axis=1))


def _cand_indices():
    s_list, e_list = [], []
    for s in range(T):
        for span in range(min(MSPAN, T - s)):
            s_list.append(s)
            e_list.append(s + span)
    return np.asarray(s_list, np.int64), np.asarray(e_list, np.int64)


def _build_bass():
    import concourse.bacc as bacc
    import concourse.tile as tile
    from concourse import mybir
    from contextlib import ExitStack

    F32 = mybir.dt.float32
    F32R = mybir.dt.float32r
    BF16 = mybir.dt.bfloat16
    F16 = mybir.dt.float16
    AF = mybir.ActivationFunctionType
    OP = mybir.AluOpType

    nc = bacc.Bacc("TRN2", target_bir_lowering=False, debug=False,
                   num_devices=N_CORES)

    doc = nc.dram_tensor("doc", [T, DD], F16, kind="ExternalInput").ap()
    amat = nc.dram_tensor("amat", [T, 2 * WIN], F16, kind="ExternalInput").ap()
    query = nc.dram_tensor("query", [LQ, EMB], F32, kind="ExternalInput").ap()
    wvec = nc.dram_tensor("wvec", [LQ, 1], F32, kind="ExternalInput").ap()
    w1t = nc.dram_tensor("w1t", [3 * DD + EMB, H4], F16, kind="ExternalInput").ap()
    w2t = nc.dram_tensor("w2t", [H4, H2], F16, kind="ExternalInput").ap()
    w3t = nc.dram_tensor("w3t", [H2, 2], F16, kind="ExternalInput").ap()
    bn1 = nc.dram_tensor("bn1", [128, 4, NHT], F32, kind="ExternalInput").ap()
    bn2 = nc.dram_tensor("bn2", [128, 4, NMT], F32, kind="ExternalInput").ap()
    y = nc.dram_tensor("y", [MSPAN, 2, NS], F32, kind="ExternalOutput").ap()

    KT1 = [(0, 128), (128, 128), (256, 48)]       # d-tiles of 304
    KTQ = [(0, 128), (128, 128), (256, 44)]       # e-tiles of 300
    KDOC = [(k, min(128, T - k)) for k in range(0, T, 128)]   # 7 k-tiles of 809

    is_f32 = [ht < N_F32_HT for ht in range(NHT)]
    NBF = NHT - N_F32_HT

    with ExitStack() as ctx:
        tc = ctx.enter_context(tile.TileContext(nc))
        const = ctx.enter_context(tc.tile_pool(name="const", bufs=1))
        work = ctx.enter_context(tc.tile_pool(name="work", bufs=4))
        zp = ctx.enter_context(tc.tile_pool(name="zp", bufs=4))
        h1p = ctx.enter_context(tc.tile_pool(name="h1p", bufs=4))
        h2p = ctx.enter_context(tc.tile_pool(name="h2p", bufs=8))
        outp = ctx.enter_context(tc.tile_pool(name="outp", bufs=3))
        ps = ctx.enter_context(tc.tile_pool(name="ps", bufs=2, space="PSUM"))
        psl2 = ctx.enter_context(tc.tile_pool(name="psl2", bufs=4, space="PSUM"))
        psl3 = ctx.enter_context(tc.tile_pool(name="psl3", bufs=2, space="PSUM"))

        # ---- small control tensors first (gpsimd queue): bn, query, U_q ----
        bn1_sb = const.tile([128, 4, NHT], F32, tag="bn1")
        bn2_sb = const.tile([128, 4, NMT], F32, tag="bn2")
        nc.gpsimd.dma_start(out=bn1_sb, in_=bn1)
        nc.gpsimd.dma_start(out=bn2_sb, in_=bn2)
        q_sb = const.tile([LQ, EMB], F32, tag="q_sb")
        nc.gpsimd.dma_start(out=q_sb, in_=query)
        wv_sb = const.tile([LQ, 1], F32, tag="wv_sb")
        nc.gpsimd.dma_start(out=wv_sb, in_=wvec)
        w1_sb = {}
        for kt, (k0, ksz) in enumerate(KTQ):
            t_ = const.tile([128, H4], F16, tag=f"w1_3_{kt}")
            nc.gpsimd.dma_start(out=t_[:ksz], in_=w1t[3 * DD + k0: 3 * DD + k0 + ksz, :])
            w1_sb[(3, kt)] = t_

        # ---- amat/doc first (F/R GEMMs are the critical path head) ----
        nk = len(KDOC)
        a_ts, d_ts = [], []
        for kt, (k0, ksz) in enumerate(KDOC):
            a_t = const.tile([128, 2 * WIN], F16, tag=f"amat{kt}")
            nc.sync.dma_start(out=a_t[:ksz], in_=amat[k0:k0 + ksz, :])
            a_ts.append(a_t)
            d_t = const.tile([128, DD], F16, tag=f"doc{kt}")
            nc.sync.dma_start(out=d_t[:ksz], in_=doc[k0:k0 + ksz, :])
            d_ts.append(d_t)

        # ---- weights to SBUF ----
        for u in range(3):
            base_row = u * DD
            for kt, (k0, ksz) in enumerate(KT1):
                t_ = const.tile([128, H4], F16, tag=f"w1_{u}_{kt}")
                nc.sync.dma_start(out=t_[:ksz], in_=w1t[base_row + k0: base_row + k0 + ksz, :])
                w1_sb[(u, kt)] = t_
        w2_sb = []
        for kt in range(NHT):
            t_ = const.tile([128, H2], F16, tag=f"w2_{kt}")
            nc.gpsimd.dma_start(out=t_, in_=w2t[kt * 128:(kt + 1) * 128, :])
            w2_sb.append(t_)
        w3_sb = []
        for mt in range(NMT):
            t_ = const.tile([128, 2], F16, tag=f"w3_{mt}")
            nc.gpsimd.dma_start(out=t_, in_=w3t[mt * 128:(mt + 1) * 128, :])
            w3_sb.append(t_)

        # ---- batchnorm scale/shift ----
        eps_sb = const.tile([128, 1], F32, tag="eps")
        nc.vector.memset(eps_sb, BN_EPS)
        zero_sb = const.tile([128, 1], F32, tag="zero")
        nc.vector.memset(zero_sb, 0.0)

        def bn_prep(src, n):
            g, b_, m, v = (src[:, i, :] for i in range(4))
            sd = const.tile([128, n], F32, tag=f"sd{n}")
            nc.scalar.activation(out=sd, in_=v, func=AF.Sqrt, bias=eps_sb, scale=1.0)
            rs = const.tile([128, n], F32, tag=f"rs{n}")
            nc.vector.reciprocal(out=rs, in_=sd)
            sc = const.tile([128, n], F32, tag=f"sc{n}")
            nc.vector.tensor_mul(sc, g, rs)
            tmp = const.tile([128, n], F32, tag=f"tmp{n}")
            nc.vector.tensor_mul(tmp, m, sc)
            sh = const.tile([128, n], F32, tag=f"sh{n}")
            nc.vector.tensor_sub(sh, b_, tmp)
            return sc, sh

        scale1, shift1 = bn_prep(bn1_sb, NHT)
        scale2, shift2 = bn_prep(bn2_sb, NMT)

        # ---- query FOFE: q1 = U_q.T @ (query.T @ wvec) as a [1,1024] row ----
        ps_qf = ps.tile([128, 3], F32, tag="ps")
        for kt, (k0, ksz) in enumerate(KTQ):
            nc.tensor.matmul(ps_qf[:ksz, kt:kt + 1], q_sb[:, k0:k0 + ksz], wv_sb[:],
                             start=True, stop=True)
        qf_sb = const.tile([128, 3], F16, tag="qf_sb")
        nc.scalar.activation(out=qf_sb, in_=ps_qf, func=AF.Copy)
        # q1 row: stationary qf [e,1], moving W1T q-rows [e, 1024] -> [1, 1024]
        ps_q1 = [ps.tile([1, 512], F32, tag="ps", name=f"ps_q1{i}") for i in range(2)]
        for kt, (k0, ksz) in enumerate(KTQ):
            for half in range(2):
                nc.tensor.matmul(ps_q1[half],
                                 qf_sb[:ksz, kt:kt + 1],
                                 w1_sb[(3, kt)][:ksz, half * 512:(half + 1) * 512],
                                 start=(kt == 0), stop=(kt == 2))
        q1_row = const.tile([1, H4], F32, tag="q1_row")
        for half in range(2):
            nc.scalar.activation(out=q1_row[:, half * 512:(half + 1) * 512],
                                 in_=ps_q1[half], func=AF.Copy)
        # transpose [1, 1024] -> [128, 8] via K=1 matmuls against ones
        ones_sb = const.tile([1, 1], F32, tag="ones")
        nc.vector.memset(ones_sb, 1.0)
        ps_q1t = ps.tile([128, NHT], F32, tag="ps")
        for ht in range(NHT):
            nc.tensor.matmul(ps_q1t[:, ht:ht + 1],
                             q1_row[:, ht * 128:(ht + 1) * 128], ones_sb[:],
                             start=True, stop=True)
        q1_sb = const.tile([128, NHT], F32, tag="q1_sb")
        nc.scalar.activation(out=q1_sb, in_=ps_q1t, func=AF.Copy)
        # q1' = q1*scale1 + shift1  (folded bias for base eviction)
        q1f_sb = const.tile([128, NHT], F32, tag="q1f_sb")
        nc.vector.tensor_mul(q1f_sb, q1_sb, scale1)
        nc.vector.tensor_add(q1f_sb, q1f_sb, shift1)

        # ---- F/R prefix GEMMs: [304, WIN] each; amat/doc resident ----
        f_sb, r_sb = [], []
        for dt, (d0, dsz) in enumerate(KT1):
            ps_fr = ps.tile([128, WIN], F32, tag="ps", name=f"ps_fr_f{dt}")
            for kt, (k0, ksz) in enumerate(KDOC):
                nc.tensor.matmul(ps_fr[:dsz], d_ts[kt][:ksz, d0:d0 + dsz],
                                 a_ts[kt][:ksz, 0:WIN], start=(kt == 0), stop=(kt == nk - 1))
            ft = const.tile([128, WIN], F16, tag=f"f{dt}")
            nc.vector.tensor_copy(ft[:dsz], ps_fr[:dsz])
            f_sb.append(ft)
            ps_fr2 = ps.tile([128, WIN], F32, tag="ps", name=f"ps_fr_r{dt}")
            for kt, (k0, ksz) in enumerate(KDOC):
                nc.tensor.matmul(ps_fr2[:dsz], d_ts[kt][:ksz, d0:d0 + dsz],
                                 a_ts[kt][:ksz, WIN:2 * WIN], start=(kt == 0), stop=(kt == nk - 1))
            rt = const.tile([128, WIN], F16, tag=f"r{dt}")
            nc.vector.tensor_copy(rt[:dsz], ps_fr2[:dsz])
            r_sb.append(rt)

        # ---- span-group assembly emitter (group g = half of the h-tiles) ----
        F32_HTS = [ht for ht in range(NHT) if is_f32[ht]]
        BF_HTS = [ht for ht in range(NHT) if not is_f32[ht]]
        BF16_ = F16
        HALF = NBF // 2
        h1_groups = {}

        def emit_group(j, g):
            cjv = float(-(ALPHA ** (j + 1)))
            zbg = zp.tile([128, HALF, NS], BF16_, tag=f"zb{g}", name=f"zb{g}_{j}")
            for i in range(HALF):
                ht = BF_HTS[g * HALF + i]
                s1 = work.tile([128, NS], BF16_, tag="s1", name=f"s1_{j}_{g}_{i}")
                nc.vector.scalar_tensor_tensor(out=s1, in0=gc_sb[ht][:, 0:NS],
                                               scalar=cjv, in1=base_sb[ht],
                                               op0=OP.mult, op1=OP.add)
                nc.vector.tensor_tensor(out=zbg[:, i, :], in0=s1,
                                        in1=e_sb[ht][:, j + 1:j + 1 + NS], op=OP.add)
            h1g = h1p.tile([128, HALF, NS], BF16_, tag=f"h1b{g}", name=f"h1b{g}_{j}")
            nc.scalar.activation(out=h1g, in_=zbg, func=AF.Relu,
                                 bias=zero_sb, scale=1.0)
            h1_groups[(j, g)] = h1g

        # ---- G GEMMs; evictions fold in scale1 (bn1 scale) ----
        # gc' = scale1*G_c, gr' = scale1*G_r, base' = scale1*G_l[0:NS] + q1'
        # E' = gc' + shift(gr'); per-tile dtype per is_f32 plan.
        gc_sb, base_sb, e_sb = [], [], []
        for ht in range(NHT):
            dt_g = F32 if is_f32[ht] else F16
            hs = slice(ht * 128, (ht + 1) * 128)
            sc_col = scale1[:, ht:ht + 1]

            ps_gc = ps.tile([128, WIN], F32, tag="ps")
            for kt, (k0, ksz) in enumerate(KT1):
                nc.tensor.matmul(ps_gc, w1_sb[(1, kt)][:ksz, hs], f_sb[kt][:ksz],
                                 start=(kt == 0), stop=(kt == 2))
            gc = const.tile([128, WIN], dt_g, tag=f"gc{ht}")
            nc.vector.tensor_scalar(out=gc, in0=ps_gc, scalar1=sc_col,
                                    scalar2=None, op0=OP.mult)
            gc_sb.append(gc)

            ps_gl = ps.tile([128, WIN], F32, tag="ps")
            for kt, (k0, ksz) in enumerate(KT1):
                nc.tensor.matmul(ps_gl, w1_sb[(0, kt)][:ksz, hs], f_sb[kt][:ksz],
                                 start=(kt == 0), stop=(kt == 2))
            bs = const.tile([128, NS], dt_g, tag=f"base{ht}")
            nc.vector.tensor_scalar(out=bs, in0=ps_gl[:, 0:NS], scalar1=sc_col,
                                    scalar2=q1f_sb[:, ht:ht + 1],
                                    op0=OP.mult, op1=OP.add)
            base_sb.append(bs)

            ps_gr = ps.tile([128, WIN], F32, tag="ps")
            for kt, (k0, ksz) in enumerate(KT1):
                nc.tensor.matmul(ps_gr, w1_sb[(2, kt)][:ksz, hs], r_sb[kt][:ksz],
                                 start=(kt == 0), stop=(kt == 2))
            gr = work.tile([128, WIN], dt_g, tag="gr")
            nc.vector.tensor_scalar(out=gr, in0=ps_gr, scalar1=sc_col,
                                    scalar2=None, op0=OP.mult)
            et = const.tile([128, WIN], dt_g, tag=f"e{ht}")
            nc.vector.tensor_tensor(out=et[:, 1:423], in0=gr[:, 2:424],
                                    in1=gc[:, 1:423], op=OP.add)
            e_sb.append(et)

        # ---- main loop over spans j ----
        for j in range(MSPAN):
            for g in range(2):
                if (j, g) not in h1_groups:
                    emit_group(j, g)
            h1bs = [h1_groups[(j, 0)], h1_groups[(j, 1)]]

            def h1_of(ht):
                i = BF_HTS.index(ht)
                return h1bs[i // HALF][:, i % HALF, :]

            h2_tiles = []
            for mt in range(NMT):
                ps_l2 = psl2.tile([128, NS], F32, tag="l2")
                ms = slice(mt * 128, (mt + 1) * 128)
                for kt in range(NHT):
                    nc.tensor.matmul(ps_l2, w2_sb[kt][:, ms], h1_of(kt),
                                     start=(kt == 0), stop=(kt == NHT - 1))
                h2 = h2p.tile([128, NS], BF16_, tag="h2")
                nc.scalar.activation(out=h2, in_=ps_l2, func=AF.Relu,
                                     bias=shift2[:, mt:mt + 1],
                                     scale=scale2[:, mt:mt + 1])
                h2_tiles.append(h2)
            ps_l3 = psl3.tile([2, NS], F32, tag="l3")
            for mt in range(NMT):
                nc.tensor.matmul(ps_l3, w3_sb[mt][:], h2_tiles[mt][:],
                                 start=(mt == 0), stop=(mt == NMT - 1))
            o = outp.tile([2, NS], F32, tag="o")
            nc.scalar.activation(out=o, in_=ps_l3, func=AF.Copy)
            nc.sync.dma_start(out=y[j], in_=o)

    nc.compile()
    return nc


def _get_nc():
    if "nc" not in _CACHE:
        _CACHE["nc"] = _build_bass()
    return _CACHE["nc"]


def _ensure_device():
    """Probe the axon device; reset it if wedged."""
    if _CACHE.get("dev_ok"):
        return
    import jax
    import jax.numpy as jnp
    try:
        (jnp.zeros((8, 8)) + 1).block_until_ready()
    except Exception:
        import ctypes
        lib = ctypes.CDLL("/opt/axon/libaxon_pjrt.so")
        lib.axon_reset.restype = ctypes.c_int64
        jax.devices()
        lib.axon_reset()
        (jnp.zeros((8, 8)) + 1).block_until_ready()
    _CACHE["dev_ok"] = True


def _make_in_maps(inputs):
    doc_emb = np.asarray(inputs["doc_emb"], np.float32)
    query_emb = np.asarray(inputs["query_emb"], np.float32)
    w1tt = np.ascontiguousarray(np.asarray(inputs["W1"], np.float32).T).astype(np.float16)
    w2tt = np.ascontiguousarray(np.asarray(inputs["W2"], np.float32).T).astype(np.float16)
    w3tt = np.ascontiguousarray(np.asarray(inputs["W3"], np.float32).T).astype(np.float16)
    wv = _round_tf32(ALPHA ** np.arange(LQ - 1, -1, -1, dtype=np.float32))[:, None]
    bn1 = np.stack([np.asarray(inputs[k], np.float32).reshape(NHT, 128).T
                    for k in ("g1", "b1", "m1", "v1")], axis=1)
    bn2 = np.stack([np.asarray(inputs[k], np.float32).reshape(NMT, 128).T
                    for k in ("g2", "b2", "m2", "v2")], axis=1)
    amats = [_build_amat(0), _build_amat(403)]
    in_maps = []
    for core in range(N_CORES):
        b, half = core // 2, core % 2
        in_maps.append({
            "doc": doc_emb[b].astype(np.float16),
            "amat": amats[half].astype(np.float16),
            "query": _round_tf32(query_emb[b]),
            "wvec": wv,
            "w1t": w1tt,
            "w2t": w2tt,
            "w3t": w3tt,
            "bn1": np.ascontiguousarray(bn1),
            "bn2": np.ascontiguousarray(bn2),
        })
    return in_maps


def _gather(results):
    s_idx, e_idx = _CACHE.setdefault("cands", _cand_indices())
    n = len(s_idx)
    j_idx = e_idx - s_idx
    half_idx = (s_idx >= 406).astype(np.int64)
    u_idx = s_idx - 403 * half_idx
    out = np.zeros((B, n, 2), np.float32)
    for b in range(B):
        both = np.stack([results[2 * b]["y"], results[2 * b + 1]["y"]])  # [2,16,2,NS]
        out[b] = both[half_idx, j_idx, :, u_idx]
    return out


def _run(inputs, trace=False):
    from concourse import bass_utils
    _ensure_device()
    nc = _get_nc()
    in_maps = _make_in_maps(inputs)
    res = bass_utils.run_bass_kernel_spmd(nc, in_maps,
                                          core_ids=list(range(N_CORES)),
                                          trace=trace)
    return _gather(res.results), res


def kernel(**inputs) -> np.ndarray:
    out, _ = _run(inputs, trace=False)
    return out



# revision 2
# speedup vs baseline: 1.1801x; 1.1801x over previous
"""FOFEReader Trainium2 kernel: 8-core SPMD (batch x s-half sharding).

Math (per batch b, candidate (s, e=s+j), j<16):
  F[t] = sum_{k<=t} a^(t-k) doc[k]      (prefix FOFE),  R[t] = sum_{k>=t} a^(k-t) doc[k]
  x = [F[s-1] | F[s+j] - a^(j+1) F[s-1] | R[s+j+1] | qf]
  out = (relu(bn2(relu(bn1(x @ W1.T)) @ W2.T)) @ W3.T)
Reformulated so the 1212-dim GEMM is shared across the 16 spans j:
  G_u = U_u @ F (u in {l,c}), G_r = U_r @ R   with W1.T = [U_l U_c U_r U_q] row blocks
  z1[s,j] = (G_l[s-1] + q1) + (G_c[s+j] + G_r[s+j+1]) - a^(j+1) G_c[s-1]
The bn1 scale is folded into the G tensors at PSUM eviction (ScalarE per-partition
scale), so the span loop is two adds + a scalar-multiply + a plain relu per tile.
F/R and G GEMMs run in fp32r (TF32); the big 1024x512 layer-2 GEMM runs in bf16
(full PE rate, half LDWEIGHTS cost). The span-loop elementwise chain runs in bf16
on VectorE for 6 of 8 h-tiles and in fp32 on GpSimd for the other 2 (GpSimd is
~2x slower and unusable for bf16, but otherwise idle).
Each core handles one batch and one half of the s range (406 starts + halo).
"""
import os
import sys

for _p in ("/opt/trn_rl_repo", "/root/.axon_site/_ro/trn_rl_repo"):
    if os.path.isdir(_p) and _p not in sys.path:
        sys.path.insert(0, _p)
        break

import numpy as np

T = 809
MSPAN = 16
B = 4
ALPHA = 0.9
NS = 406          # s-starts per core (even: f32r matmul needs even free dim)
WIN = 424         # t window per core: t = s_lo-1 + i, i in [0, 424)
DD = 304
EMB = 300
LQ = 30
H4 = 1024
H2 = 512
BN_EPS = 1e-5
N_CORES = 8
NHT = H4 // 128   # 8
NMT = H2 // 128   # 4
N_F32_HT = 0      # h-tiles assembled in fp32 on GpSimd (rest: bf16 on VectorE)

_CACHE = {}


def _round_tf32(a):
    a = np.ascontiguousarray(a, dtype=np.float32)
    return (a.view(np.uint32) & np.uint32(0xFFFFE000)).view(np.float32)


def _build_amat(s_lo):
    """[809, 2*WIN] fp32: cols 0..WIN-1 = forward-FOFE operator columns for
    t=s_lo-1+i (A^T slice), cols WIN.. = reverse. Out-of-range t -> zero col."""
    t_idx = s_lo - 1 + np.arange(WIN)
    kv = np.arange(T)[:, None]
    tv = t_idx[None, :]
    valid = ((t_idx >= 0) & (t_idx <= T - 1))[None, :]
    af = np.where((kv <= tv) & valid, ALPHA ** np.maximum(tv - kv, 0), 0.0)
    ar = np.where((kv >= tv) & valid, ALPHA ** np.maximum(kv - tv, 0), 0.0)
    return _round_tf32(np.concatenate([af, ar], axis=1))


def _cand_indices():
    s_list, e_list = [], []
    for s in range(T):
        for span in range(min(MSPAN, T - s)):
            s_list.append(s)
            e_list.append(s + span)
    return np.asarray(s_list, np.int64), np.asarray(e_list, np.int64)


def _build_bass():
    import concourse.bacc as bacc
    import concourse.tile as tile
    from concourse import mybir
    from contextlib import ExitStack

    F32 = mybir.dt.float32
    F32R = mybir.dt.float32r
    BF16 = mybir.dt.bfloat16
    F16 = mybir.dt.float16
    AF = mybir.ActivationFunctionType
    OP = mybir.AluOpType

    nc = bacc.Bacc("TRN2", target_bir_lowering=False, debug=False,
                   num_devices=N_CORES)

    doc = nc.dram_tensor("doc", [T, DD], F16, kind="ExternalInput").ap()
    amat = nc.dram_tensor("amat", [T, 2 * WIN], F16, kind="ExternalInput").ap()
    query = nc.dram_tensor("query", [LQ, EMB], F32, kind="ExternalInput").ap()
    wvec = nc.dram_tensor("wvec", [LQ, 1], F32, kind="ExternalInput").ap()
    w1t = nc.dram_tensor("w1t", [3 * DD + EMB, H4], F16, kind="ExternalInput").ap()
    w2t = nc.dram_tensor("w2t", [H4, H2], BF16, kind="ExternalInput").ap()
    w3t = nc.dram_tensor("w3t", [H2, 2], BF16, kind="ExternalInput").ap()
    bn1 = nc.dram_tensor("bn1", [128, 4, NHT], F32, kind="ExternalInput").ap()
    bn2 = nc.dram_tensor("bn2", [128, 4, NMT], F32, kind="ExternalInput").ap()
    y = nc.dram_tensor("y", [MSPAN, 2, NS], F32, kind="ExternalOutput").ap()

    KT1 = [(0, 128), (128, 128), (256, 48)]       # d-tiles of 304
    KTQ = [(0, 128), (128, 128), (256, 44)]       # e-tiles of 300
    KDOC = [(k, min(128, T - k)) for k in range(0, T, 128)]   # 7 k-tiles of 809

    is_f32 = [ht < N_F32_HT for ht in range(NHT)]
    NBF = NHT - N_F32_HT

    with ExitStack() as ctx:
        tc = ctx.enter_context(tile.TileContext(nc))
        const = ctx.enter_context(tc.tile_pool(name="const", bufs=1))
        work = ctx.enter_context(tc.tile_pool(name="work", bufs=4))
        zp = ctx.enter_context(tc.tile_pool(name="zp", bufs=4))
        h1p = ctx.enter_context(tc.tile_pool(name="h1p", bufs=4))
        h2p = ctx.enter_context(tc.tile_pool(name="h2p", bufs=8))
        outp = ctx.enter_context(tc.tile_pool(name="outp", bufs=3))
        ps = ctx.enter_context(tc.tile_pool(name="ps", bufs=2, space="PSUM"))
        psl2 = ctx.enter_context(tc.tile_pool(name="psl2", bufs=4, space="PSUM"))
        psl3 = ctx.enter_context(tc.tile_pool(name="psl3", bufs=2, space="PSUM"))

        # ---- small control tensors first (gpsimd queue): bn, query, U_q ----
        bn1_sb = const.tile([128, 4, NHT], F32, tag="bn1")
        bn2_sb = const.tile([128, 4, NMT], F32, tag="bn2")
        nc.gpsimd.dma_start(out=bn1_sb, in_=bn1)
        nc.gpsimd.dma_start(out=bn2_sb, in_=bn2)
        q_sb = const.tile([LQ, EMB], F32, tag="q_sb")
        nc.gpsimd.dma_start(out=q_sb, in_=query)
        wv_sb = const.tile([LQ, 1], F32, tag="wv_sb")
        nc.gpsimd.dma_start(out=wv_sb, in_=wvec)
        w1_sb = {}
        for kt, (k0, ksz) in enumerate(KTQ):
            t_ = const.tile([128, H4], F16, tag=f"w1_3_{kt}")
            nc.gpsimd.dma_start(out=t_[:ksz], in_=w1t[3 * DD + k0: 3 * DD + k0 + ksz, :])
            w1_sb[(3, kt)] = t_

        # ---- amat/doc first (F/R GEMMs are the critical path head) ----
        nk = len(KDOC)
        a_ts, d_ts = [], []
        for kt, (k0, ksz) in enumerate(KDOC):
            a_t = const.tile([128, 2 * WIN], F16, tag=f"amat{kt}")
            nc.sync.dma_start(out=a_t[:ksz], in_=amat[k0:k0 + ksz, :])
            a_ts.append(a_t)
            d_t = const.tile([128, DD], F16, tag=f"doc{kt}")
            nc.sync.dma_start(out=d_t[:ksz], in_=doc[k0:k0 + ksz, :])
            d_ts.append(d_t)

        # ---- weights to SBUF ----
        for u in range(3):
            base_row = u * DD
            for kt, (k0, ksz) in enumerate(KT1):
                t_ = const.tile([128, H4], F16, tag=f"w1_{u}_{kt}")
                nc.sync.dma_start(out=t_[:ksz], in_=w1t[base_row + k0: base_row + k0 + ksz, :])
                w1_sb[(u, kt)] = t_
        w2_sb = []
        for kt in range(NHT):
            t_ = const.tile([128, H2], BF16, tag=f"w2_{kt}")
            nc.gpsimd.dma_start(out=t_, in_=w2t[kt * 128:(kt + 1) * 128, :])
            w2_sb.append(t_)
        w3_sb = []
        for mt in range(NMT):
            t_ = const.tile([128, 2], BF16, tag=f"w3_{mt}")
            nc.gpsimd.dma_start(out=t_, in_=w3t[mt * 128:(mt + 1) * 128, :])
            w3_sb.append(t_)

        # ---- batchnorm scale/shift ----
        eps_sb = const.tile([128, 1], F32, tag="eps")
        nc.vector.memset(eps_sb, BN_EPS)
        zero_sb = const.tile([128, 1], F32, tag="zero")
        nc.vector.memset(zero_sb, 0.0)

        def bn_prep(src, n):
            g, b_, m, v = (src[:, i, :] for i in range(4))
            sd = const.tile([128, n], F32, tag=f"sd{n}")
            nc.scalar.activation(out=sd, in_=v, func=AF.Sqrt, bias=eps_sb, scale=1.0)
            rs = const.tile([128, n], F32, tag=f"rs{n}")
            nc.vector.reciprocal(out=rs, in_=sd)
            sc = const.tile([128, n], F32, tag=f"sc{n}")
            nc.vector.tensor_mul(sc, g, rs)
            tmp = const.tile([128, n], F32, tag=f"tmp{n}")
            nc.vector.tensor_mul(tmp, m, sc)
            sh = const.tile([128, n], F32, tag=f"sh{n}")
            nc.vector.tensor_sub(sh, b_, tmp)
            return sc, sh

        scale1, shift1 = bn_prep(bn1_sb, NHT)
        scale2, shift2 = bn_prep(bn2_sb, NMT)

        # ---- query FOFE: q1 = U_q.T @ (query.T @ wvec) as a [1,1024] row ----
        ps_qf = ps.tile([128, 3], F32, tag="ps")
        for kt, (k0, ksz) in enumerate(KTQ):
            nc.tensor.matmul(ps_qf[:ksz, kt:kt + 1], q_sb[:, k0:k0 + ksz], wv_sb[:],
                             start=True, stop=True)
        qf_sb = const.tile([128, 3], F16, tag="qf_sb")
        nc.scalar.activation(out=qf_sb, in_=ps_qf, func=AF.Copy)
        # q1 row: stationary qf [e,1], moving W1T q-rows [e, 1024] -> [1, 1024]
        ps_q1 = [ps.tile([1, 512], F32, tag="ps", name=f"ps_q1{i}") for i in range(2)]
        for kt, (k0, ksz) in enumerate(KTQ):
            for half in range(2):
                nc.tensor.matmul(ps_q1[half],
                                 qf_sb[:ksz, kt:kt + 1],
                                 w1_sb[(3, kt)][:ksz, half * 512:(half + 1) * 512],
                                 start=(kt == 0), stop=(kt == 2))
        q1_row = const.tile([1, H4], F32, tag="q1_row")
        for half in range(2):
            nc.scalar.activation(out=q1_row[:, half * 512:(half + 1) * 512],
                                 in_=ps_q1[half], func=AF.Copy)
        # transpose [1, 1024] -> [128, 8] via K=1 matmuls against ones
        ones_sb = const.tile([1, 1], F32, tag="ones")
        nc.vector.memset(ones_sb, 1.0)
        ps_q1t = ps.tile([128, NHT], F32, tag="ps")
        for ht in range(NHT):
            nc.tensor.matmul(ps_q1t[:, ht:ht + 1],
                             q1_row[:, ht * 128:(ht + 1) * 128], ones_sb[:],
                             start=True, stop=True)
        q1_sb = const.tile([128, NHT], F32, tag="q1_sb")
        nc.scalar.activation(out=q1_sb, in_=ps_q1t, func=AF.Copy)
        # q1' = q1*scale1 + shift1  (folded bias for base eviction)
        q1f_sb = const.tile([128, NHT], F32, tag="q1f_sb")
        nc.vector.tensor_mul(q1f_sb, q1_sb, scale1)
        nc.vector.tensor_add(q1f_sb, q1f_sb, shift1)

        # ---- F/R prefix GEMMs: [304, WIN] each; amat/doc resident ----
        f_sb, r_sb = [], []
        for dt, (d0, dsz) in enumerate(KT1):
            ps_fr = ps.tile([128, WIN], F32, tag="ps", name=f"ps_fr_f{dt}")
            for kt, (k0, ksz) in enumerate(KDOC):
                nc.tensor.matmul(ps_fr[:dsz], d_ts[kt][:ksz, d0:d0 + dsz],
                                 a_ts[kt][:ksz, 0:WIN], start=(kt == 0), stop=(kt == nk - 1))
            ft = const.tile([128, WIN], F16, tag=f"f{dt}")
            nc.vector.tensor_copy(ft[:dsz], ps_fr[:dsz])
            f_sb.append(ft)
            ps_fr2 = ps.tile([128, WIN], F32, tag="ps", name=f"ps_fr_r{dt}")
            for kt, (k0, ksz) in enumerate(KDOC):
                nc.tensor.matmul(ps_fr2[:dsz], d_ts[kt][:ksz, d0:d0 + dsz],
                                 a_ts[kt][:ksz, WIN:2 * WIN], start=(kt == 0), stop=(kt == nk - 1))
            rt = const.tile([128, WIN], F16, tag=f"r{dt}")
            nc.vector.tensor_copy(rt[:dsz], ps_fr2[:dsz])
            r_sb.append(rt)

        # ---- span-group assembly emitter (group g = half of the h-tiles) ----
        F32_HTS = [ht for ht in range(NHT) if is_f32[ht]]
        BF_HTS = [ht for ht in range(NHT) if not is_f32[ht]]
        BF16_ = BF16
        HALF = NBF // 2
        h1_groups = {}

        def emit_group(j, g):
            cjv = float(-(ALPHA ** (j + 1)))
            zbg = zp.tile([128, HALF, NS], BF16_, tag=f"zb{g}", name=f"zb{g}_{j}")
            for i in range(HALF):
                ht = BF_HTS[g * HALF + i]
                s1 = work.tile([128, NS], BF16_, tag="s1", name=f"s1_{j}_{g}_{i}")
                nc.vector.scalar_tensor_tensor(out=s1, in0=gc_sb[ht][:, 0:NS],
                                               scalar=cjv, in1=base_sb[ht],
                                               op0=OP.mult, op1=OP.add)
                nc.vector.tensor_tensor(out=zbg[:, i, :], in0=s1,
                                        in1=e_sb[ht][:, j + 1:j + 1 + NS], op=OP.add)
            h1g = h1p.tile([128, HALF, NS], BF16_, tag=f"h1b{g}", name=f"h1b{g}_{j}")
            nc.scalar.activation(out=h1g, in_=zbg, func=AF.Relu,
                                 bias=zero_sb, scale=1.0)
            h1_groups[(j, g)] = h1g

        # ---- G GEMMs; evictions fold in scale1 (bn1 scale) ----
        # gc' = scale1*G_c, gr' = scale1*G_r, base' = scale1*G_l[0:NS] + q1'
        # E' = gc' + shift(gr'); per-tile dtype per is_f32 plan.
        gc_sb, base_sb, e_sb = [], [], []
        for ht in range(NHT):
            dt_g = F32 if is_f32[ht] else BF16
            hs = slice(ht * 128, (ht + 1) * 128)
            sc_col = scale1[:, ht:ht + 1]

            ps_gc = ps.tile([128, WIN], F32, tag="ps")
            for kt, (k0, ksz) in enumerate(KT1):
                nc.tensor.matmul(ps_gc, w1_sb[(1, kt)][:ksz, hs], f_sb[kt][:ksz],
                                 start=(kt == 0), stop=(kt == 2))
            gc = const.tile([128, WIN], dt_g, tag=f"gc{ht}")
            nc.vector.tensor_scalar(out=gc, in0=ps_gc, scalar1=sc_col,
                                    scalar2=None, op0=OP.mult)
            gc_sb.append(gc)

            ps_gl = ps.tile([128, WIN], F32, tag="ps")
            for kt, (k0, ksz) in enumerate(KT1):
                nc.tensor.matmul(ps_gl, w1_sb[(0, kt)][:ksz, hs], f_sb[kt][:ksz],
                                 start=(kt == 0), stop=(kt == 2))
            bs = const.tile([128, NS], dt_g, tag=f"base{ht}")
            nc.vector.tensor_scalar(out=bs, in0=ps_gl[:, 0:NS], scalar1=sc_col,
                                    scalar2=q1f_sb[:, ht:ht + 1],
                                    op0=OP.mult, op1=OP.add)
            base_sb.append(bs)

            ps_gr = ps.tile([128, WIN], F32, tag="ps")
            for kt, (k0, ksz) in enumerate(KT1):
                nc.tensor.matmul(ps_gr, w1_sb[(2, kt)][:ksz, hs], r_sb[kt][:ksz],
                                 start=(kt == 0), stop=(kt == 2))
            gr = work.tile([128, WIN], dt_g, tag="gr")
            nc.vector.tensor_scalar(out=gr, in0=ps_gr, scalar1=sc_col,
                                    scalar2=None, op0=OP.mult)
            et = const.tile([128, WIN], dt_g, tag=f"e{ht}")
            nc.vector.tensor_tensor(out=et[:, 1:423], in0=gr[:, 2:424],
                                    in1=gc[:, 1:423], op=OP.add)
            e_sb.append(et)

        # ---- main loop over spans j ----
        for j in range(MSPAN):
            for g in range(2):
                if (j, g) not in h1_groups:
                    emit_group(j, g)
            h1bs = [h1_groups[(j, 0)], h1_groups[(j, 1)]]

            def h1_of(ht):
                i = BF_HTS.index(ht)
                return h1bs[i // HALF][:, i % HALF, :]

            h2_tiles = []
            for mt in range(NMT):
                ps_l2 = psl2.tile([128, NS], F32, tag="l2")
                ms = slice(mt * 128, (mt + 1) * 128)
                for kt in range(NHT):
                    nc.tensor.matmul(ps_l2, w2_sb[kt][:, ms], h1_of(kt),
                                     start=(kt == 0), stop=(kt == NHT - 1))
                h2 = h2p.tile([128, NS], BF16_, tag="h2")
                nc.scalar.activation(out=h2, in_=ps_l2, func=AF.Relu,
                                     bias=shift2[:, mt:mt + 1],
                                     scale=scale2[:, mt:mt + 1])
                h2_tiles.append(h2)
            ps_l3 = psl3.tile([2, NS], F32, tag="l3")
            for mt in range(NMT):
                nc.tensor.matmul(ps_l3, w3_sb[mt][:], h2_tiles[mt][:],
                                 start=(mt == 0), stop=(mt == NMT - 1))
            o = outp.tile([2, NS], F32, tag="o")
            nc.scalar.activation(out=o, in_=ps_l3, func=AF.Copy)
            nc.sync.dma_start(out=y[j], in_=o)

    nc.compile()
    return nc


def _get_nc():
    if "nc" not in _CACHE:
        _CACHE["nc"] = _build_bass()
    return _CACHE["nc"]


def _ensure_device():
    """Probe the axon device; reset it if wedged."""
    if _CACHE.get("dev_ok"):
        return
    import jax
    import jax.numpy as jnp
    try:
        (jnp.zeros((8, 8)) + 1).block_until_ready()
    except Exception:
        import ctypes
        lib = ctypes.CDLL("/opt/axon/libaxon_pjrt.so")
        lib.axon_reset.restype = ctypes.c_int64
        jax.devices()
        lib.axon_reset()
        (jnp.zeros((8, 8)) + 1).block_until_ready()
    _CACHE["dev_ok"] = True


def _make_in_maps(inputs):
    doc_emb = np.asarray(inputs["doc_emb"], np.float32)
    query_emb = np.asarray(inputs["query_emb"], np.float32)
    w1tt = np.ascontiguousarray(np.asarray(inputs["W1"], np.float32).T).astype(np.float16)
    import ml_dtypes
    w2tt = np.ascontiguousarray(np.asarray(inputs["W2"], np.float32).T).astype(ml_dtypes.bfloat16)
    w3tt = np.ascontiguousarray(np.asarray(inputs["W3"], np.float32).T).astype(ml_dtypes.bfloat16)
    wv = _round_tf32(ALPHA ** np.arange(LQ - 1, -1, -1, dtype=np.float32))[:, None]
    bn1 = np.stack([np.asarray(inputs[k], np.float32).reshape(NHT, 128).T
                    for k in ("g1", "b1", "m1", "v1")], axis=1)
    bn2 = np.stack([np.asarray(inputs[k], np.float32).reshape(NMT, 128).T
                    for k in ("g2", "b2", "m2", "v2")], axis=1)
    amats = [_build_amat(0), _build_amat(403)]
    in_maps = []
    for core in range(N_CORES):
        b, half = core // 2, core % 2
        in_maps.append({
            "doc": doc_emb[b].astype(np.float16),
            "amat": amats[half].astype(np.float16),
            "query": _round_tf32(query_emb[b]),
            "wvec": wv,
            "w1t": w1tt,
            "w2t": w2tt,
            "w3t": w3tt,
            "bn1": np.ascontiguousarray(bn1),
            "bn2": np.ascontiguousarray(bn2),
        })
    return in_maps


def _gather(results):
    s_idx, e_idx = _CACHE.setdefault("cands", _cand_indices())
    n = len(s_idx)
    j_idx = e_idx - s_idx
    half_idx = (s_idx >= 406).astype(np.int64)
    u_idx = s_idx - 403 * half_idx
    out = np.zeros((B, n, 2), np.float32)
    for b in range(B):
        both = np.stack([results[2 * b]["y"], results[2 * b + 1]["y"]])  # [2,16,2,NS]
        out[b] = both[half_idx, j_idx, :, u_idx]
    return out


def _run(inputs, trace=False):
    from concourse import bass_utils
    _ensure_device()
    nc = _get_nc()
    in_maps = _make_in_maps(inputs)
    res = bass_utils.run_bass_kernel_spmd(nc, in_maps,
                                          core_ids=list(range(N_CORES)),
                                          trace=trace)
    return _gather(res.results), res


def kernel(**inputs) -> np.ndarray:
    out, _ = _run(inputs, trace=False)
    return out

